# revision 32
# baseline (speedup 1.0000x reference)
"""GroupedQueryAttentionLayer on 8 trn2 NeuronCores (Bass/Tile, SPMD).

Sharding: data-parallel over query rows; no collectives. Core i handles
batch b = i//4, query rows q0 = (i%4)*512 .. +512. Each core recomputes
its batch's K/V projection (cheap vs. attention); outputs are disjoint
row-slices of the final [2, 2048, 1024].

Host-prepared per-core layouts (transposes/casts are part of sharding):
  XT   [128, 8, 2048] fp8e4 : X[b].T k-chunked (XT[p,c,t] = X[b,t,c*128+p])
  XTQ  [128, 8,  512] fp8e4 : XT columns q0..q0+512 (this core's queries)
  XRES [128, 4, 1024] f32   : residual rows (exact fp32)
  WQ/WP [128, 8, 1024] fp8e4, WK/WV [128, 8, 256] fp8e4 (k-chunked weights)
Output OUT [4, 128, 1024] f32, OUT[sc, p, :] = row q0 + sc*128 + p.

Kernel: softmax kept with t on PSUM partitions.
- Projections (Q/K/V/post) run as fp8 DoubleRow matmuls: each instruction
  contracts two 128-row k-chunks (lhsT/rhs laid out [128, 2, m]) at 2 fp8
  rows/clk, halving PE time vs bf16.
- Scores stay bf16 for accuracy but are ROW-TILED: per head pair, head 2c
  occupies PE rows 0-63 (K=64 via tile_position (0,0)) and head 2c+1 rows
  64-127 ((64,0)); the two matmuls execute concurrently on the array
  halves, halving score PE time. ktz keeps K_g at both parities so the
  64-row slices line up with qt's head layout.
- V carries a ones column so the PV DoubleRow matmul emits the softmax
  denominator as row 64. exp on ScalarE from 2-bank PSUM blocks straight
  to fp8 (no max subtraction: |scores| <= ~4 by construction).
- Head pairs are software-pipelined: scores/exp of pair c run while pair
  c-1's PV-finish, SBUF-evict, partition-spread reciprocal,
  broadcast-matmul normalize, and post-projection fill fixed slots. The
  post output accumulates in SBUF; the residual-add streams out.
"""

import math

import numpy as np
import ml_dtypes

BF16 = ml_dtypes.bfloat16
FP8 = ml_dtypes.float8_e4m3  # TRN float8e4 (e4m3, max 240)

B, S, D = 2, 2048, 1024
HEADS, GROUPS, E = 16, 4, 64
HPG = HEADS // GROUPS
NCORES = 8
CORES_PER_BATCH = NCORES // B
SLOC = B * S // NCORES
SCALE = 1.0 / math.sqrt(E)

_prog_cache = {}


def _build_program():
    from contextlib import ExitStack

    import concourse.bacc as bacc
    import concourse.tile as tile
    from concourse import mybir

    f32 = mybir.dt.float32
    b16 = mybir.dt.bfloat16
    f8 = mybir.dt.float8e4
    Exp = mybir.ActivationFunctionType.Exp
    DR = mybir.MatmulPerfMode.DoubleRow

    nc = bacc.Bacc("TRN2", target_bir_lowering=False)

    xt_d = nc.dram_tensor("XT", [128, 8, S], f8, kind="ExternalInput")
    xtq_d = nc.dram_tensor("XTQ", [128, 8, SLOC], f8, kind="ExternalInput")
    xres_d = nc.dram_tensor("XRES", [128, 4, D], f32, kind="ExternalInput")
    wq_d = nc.dram_tensor("WQ", [128, 8, 1024], f8, kind="ExternalInput")
    wk_d = nc.dram_tensor("WK", [128, 8, 256], f8, kind="ExternalInput")
    wv_d = nc.dram_tensor("WV", [128, 8, 256], f8, kind="ExternalInput")
    wp_d = nc.dram_tensor("WP", [128, 8, 1024], f8, kind="ExternalInput")
    out_d = nc.dram_tensor("OUT", [4, 128, D], f32, kind="ExternalOutput")

    with tile.TileContext(nc) as tc, ExitStack() as ctx:
        consts = ctx.enter_context(tc.tile_pool(name="consts", bufs=1))
        work = ctx.enter_context(tc.tile_pool(name="work", bufs=2))
        # PSUM (8 banks): scores 2x2 + pv 2x1 + pp(proj/post/bcast) 2x1
        psA = ctx.enter_context(tc.tile_pool(name="psA", bufs=2, space="PSUM"))
        psS = ctx.enter_context(tc.tile_pool(name="psS", bufs=2, space="PSUM"))
        psV = ctx.enter_context(tc.tile_pool(name="psV", bufs=1, space="PSUM"))

        xt = consts.tile([128, 8, S], f8)
        xtq = consts.tile([128, 8, SLOC], f8)
        xres = consts.tile([128, 4, D], f32)
        wq = consts.tile([128, 8, 1024], f8)
        wk = consts.tile([128, 8, 256], f8)
        wv = consts.tile([128, 8, 256], f8)
        wp = consts.tile([128, 8, 1024], f8)
        ktz = consts.tile([128, 8, S], b16)  # slot 2g+j holds K_g at parity j rows
        # last dim padded 65->68 so the t-chunk-pair stride (4*68=272B) meets
        # the DoubleRow ldweights step%16==0 restriction
        vpr = consts.tile([128, 16, 4, E + 4], f8)
        qt = consts.tile([128, 8, SLOC], b16)
        atn = consts.tile([128, 8, SLOC], f8)
        pacc = consts.tile([128, 4, D], f32)  # streamed post accumulation
        e64 = consts.tile([128, 128], b16)  # row 64 = 1, else 0 (K=128 bcast)
        rbe = consts.tile([128, 512], b16)  # recip staging, rows != 64 stay 0
        rbo = consts.tile([128, 512], b16)
        warm = consts.tile([128, 8], f32)

        nc.vector.memset(e64[:], 0.0)
        nc.vector.memset(e64[64:65, :], 1.0)
        nc.vector.memset(rbe[:], 0.0)
        nc.vector.memset(rbo[:], 0.0)
        nc.vector.memset(vpr[:, :, :, E:E + 1], 1.0)  # ones column only
        nc.vector.memset(warm[:], 0.0)
        nc.scalar.activation(warm[:], warm[:], Exp)  # exp table preload

        # All input DMAs share one ~358GB/s HBM stream: issue strictly in
        # first-use order. Late-use bulk (wq tail / wp / xres) is triggered
        # from inside pair 0 so it can't jump ahead of the xt chunks.
        nc.sync.dma_start(out=wk[:], in_=wk_d[:])
        nc.sync.dma_start(out=wv[:], in_=wv_d[:])
        nc.sync.dma_start(out=xt[:, :, 0:512], in_=xt_d[:, :, 0:512])
        nc.sync.dma_start(out=wq[:, :, 0:256], in_=wq_d[:, :, 0:256])
        nc.sync.dma_start(out=xtq[:], in_=xtq_d[:])
        for t4 in range(1, 4):
            sl = slice(t4 * 512, (t4 + 1) * 512)
            nc.sync.dma_start(out=xt[:, :, sl], in_=xt_d[:, :, sl])

        def late_loads(tcb):
            # triggered from the Scalar strict-FIFO queue (behind pair-0's
            # exps) so the transfers stay behind the critical loads in the
            # shared HBM stream; gpsimd triggers get hoisted to t=0 (its 8
            # Q7 queues run independently)
            if tcb == 0:
                nc.scalar.dma_start(out=wq[:, :, 256:1024], in_=wq_d[:, :, 256:1024])
            elif tcb == 2:
                nc.scalar.dma_start(out=wp[:], in_=wp_d[:])
            elif tcb == 4:
                nc.scalar.dma_start(out=xres[:], in_=xres_d[:])

        def k_proj_tb(ec, tb):
            ps = psA.tile([128, 512], f32, tag="pp")
            for ki in range(4):
                nc.tensor.matmul(
                    ps[:],
                    lhsT=wk[:, 2 * ki:2 * ki + 2, ec * 128:(ec + 1) * 128],
                    rhs=xt[:, 2 * ki:2 * ki + 2, tb * 512:(tb + 1) * 512],
                    start=(ki == 0),
                    stop=(ki == 3),
                    perf_mode=DR,
                )
            sl = slice(tb * 512, (tb + 1) * 512)
            ga, gb = 2 * ec, 2 * ec + 1
            nc.vector.tensor_copy(ktz[0:64, 2 * ga, sl], ps[0:64, :])
            nc.vector.tensor_copy(ktz[64:128, 2 * gb + 1, sl], ps[64:128, :])
            nc.gpsimd.dma_start(out=ktz[64:128, 2 * ga + 1, sl], in_=ktz[0:64, 2 * ga, sl])
            nc.gpsimd.dma_start(out=ktz[0:64, 2 * gb, sl], in_=ktz[64:128, 2 * gb + 1, sl])

        def q_proj(hc):
            ps = psA.tile([128, 512], f32, tag="pp")
            for ki in range(4):
                nc.tensor.matmul(
                    ps[:],
                    lhsT=wq[:, 2 * ki:2 * ki + 2, hc * 128:(hc + 1) * 128],
                    rhs=xtq[:, 2 * ki:2 * ki + 2, :],
                    start=(ki == 0),
                    stop=(ki == 3),
                    perf_mode=DR,
                )
            nc.vector.tensor_scalar_mul(qt[:, hc, :], ps, SCALE)

        def v_proj_tcc(tcc):
            ps = psA.tile([128, 256], f32, tag="pp")
            for ki in range(4):
                nc.tensor.matmul(
                    ps[:],
                    lhsT=xt[:, 2 * ki:2 * ki + 2, tcc * 128:(tcc + 1) * 128],
                    rhs=wv[:, 2 * ki:2 * ki + 2, :],
                    start=(ki == 0),
                    stop=(ki == 3),
                    perf_mode=DR,
                )
            nc.vector.tensor_copy(
                vpr[:, tcc, :, 0:E], ps.rearrange("p (g e) -> p g e", g=4)
            )

        state = {}  # live psV tiles per pair: c -> (pve, pvo)

        def pv_mm(c, tp):
            """PV DoubleRow step tp (t-chunks 2tp, 2tp+1) for both heads."""
            g = c // 2
            if tp == 0:
                state[c] = (
                    psV.tile([E + 1, 512], f32, tag="pve", name="pve"),
                    psV.tile([E + 1, 512], f32, tag="pvo", name="pvo"),
                )
            pve, pvo = state[c]
            exb = exbs.pop((c, tp))
            for j, pv in ((0, pve), (1, pvo)):
                nc.tensor.matmul(
                    pv[:],
                    lhsT=vpr[:, 2 * tp:2 * tp + 2, g, 0:E + 1],
                    rhs=exb[:, :, j, :],
                    start=(tp == 0),
                    stop=(tp == 7),
                    perf_mode=DR,
                )

        aun = {}

        def pv_evict(c, drain=False):
            """Copy A' to SBUF right after PV stop so the PSUM slots free
            early; the normalize chain then runs from SBUF. In the drain
            (no exp left) ScalarE is idle: use it for the casts."""
            pve, pvo = state.pop(c)
            te = work.tile([65, 512], b16, tag="aune", name="aune")
            to = work.tile([65, 512], b16, tag="auno", name="auno")
            if drain:
                nc.scalar.copy(te[:], pve[:])
                nc.scalar.copy(to[:], pvo[:])
            else:
                nc.vector.tensor_copy(te[:], pve[:])
                nc.vector.tensor_copy(to[:], pvo[:])
            aun[c] = (te, to)

        def recips(c, drain=False):
            te, to = aun[c]
            if drain:
                # serial chain anyway: 1-lane direct reciprocal avoids the
                # two gpsimd-DMA latency hops
                with nc.allow_low_precision(reason="bf16 softmax recip"):
                    nc.vector.reciprocal(rbe[64:65, :], te[64:65, :])
                    nc.vector.reciprocal(rbo[64:65, :], to[64:65, :])
                return
            for t, rb in ((te, rbe), (to, rbo)):
                # spread the 512 denominators over 64 partitions so the DVE
                # reciprocal runs at 8 elements/lane instead of 512
                dsp = work.tile([64, 8], b16, tag="dsp")
                nc.gpsimd.dma_start(
                    out=dsp[:, None, :],
                    in_=t[64:65, :].rearrange("p (a b) -> p a b", a=64),
                )
                rsp = work.tile([64, 8], b16, tag="rsp")
                with nc.allow_low_precision(reason="bf16 softmax recip"):
                    nc.vector.reciprocal(rsp[:], dsp[:])
                nc.gpsimd.dma_start(
                    out=rb[64:65, :].rearrange("p (a b) -> p a b", a=64),
                    in_=rsp[:, None, :],
                )

        def norm_head(c, j):
            te, to = aun[c]
            t, rb = (te, rbe) if j == 0 else (to, rbo)
            bc = psA.tile([128, 512], f32, tag="pp")
            nc.tensor.matmul(bc[:], lhsT=e64[:], rhs=rb[:], start=True, stop=True)
            bcs = work.tile([64, 512], b16, tag="bc")
            nc.vector.tensor_copy(bcs[:], bc[0:64, :])
            # SBUF-only multiplies run on GpSimd to keep the Vector queue short
            if j == 0:
                nc.gpsimd.tensor_mul(atn[0:64, c, :], t[0:64, :], bcs[:])
            else:
                so = work.tile([64, 512], f8, tag="so")
                nc.gpsimd.tensor_mul(so[:], t[0:64, :], bcs[:])
                nc.gpsimd.dma_start(out=atn[64:128, c, :], in_=so[:])
                aun.pop(c)

        def post_chunk2(c0, i):
            """One DoubleRow matmul: he-chunks c0,c0+1 into one PSUM tile."""
            sc, dc = i // 2, i % 2
            pp = psA.tile([128, 512], f32, tag="pp")
            nc.tensor.matmul(
                pp[:],
                lhsT=atn[:, c0:c0 + 2, sc * 128:(sc + 1) * 128],
                rhs=wp[:, c0:c0 + 2, dc * 512:(dc + 1) * 512],
                start=True,
                stop=True,
                perf_mode=DR,
            )
            dsl = slice(dc * 512, (dc + 1) * 512)
            if c0 == 0:
                nc.vector.tensor_copy(pacc[:, sc, dsl], pp[:])
            else:
                nc.vector.tensor_add(pacc[:, sc, dsl], pacc[:, sc, dsl], pp[:])

        def xres_preadd(i):
            """Fold the residual into pacc while exp still streams (pair 7)."""
            sc, dc = i // 2, i % 2
            dsl = slice(dc * 512, (dc + 1) * 512)
            nc.gpsimd.tensor_add(pacc[:, sc, dsl], pacc[:, sc, dsl], xres[:, sc, dsl])

        def post_single(cx, i):
            """Single-chunk post matmul (K=128, no DoubleRow). For chunk 6 it
            accumulates into pacc during pair 7; for chunk 7 (the drain) it
            fuses the final add and streams the output out."""
            sc, dc = i // 2, i % 2
            pp = psA.tile([128, 512], f32, tag="pp")
            nc.tensor.matmul(
                pp[:],
                lhsT=atn[:, cx, sc * 128:(sc + 1) * 128],
                rhs=wp[:, cx, dc * 512:(dc + 1) * 512],
                start=True,
                stop=True,
            )
            dsl = slice(dc * 512, (dc + 1) * 512)
            if cx == 6:
                nc.vector.tensor_add(pacc[:, sc, dsl], pacc[:, sc, dsl], pp[:])
            else:
                ores = work.tile([128, 512], f32, tag="or")
                nc.vector.tensor_add(ores[:], pacc[:, sc, dsl], pp[:])
                dq = nc.sync if i % 2 == 0 else nc.gpsimd
                dq.dma_start(out=out_d[sc, :, dsl], in_=ores[:])

        exbs = {}

        def pair_blocks(c):
            """One pipeline step: scores/exp for pair c (row-tiled 64-row
            matmuls, heads on PE halves run concurrently), trailing PV for c,
            and the tail (PV-finish, normalize, post) of pair c-1."""
            g = c // 2
            for tcb in range(8):
                if c < 8:
                    if c == 0:
                        v_proj_tcc(2 * tcb)
                        v_proj_tcc(2 * tcb + 1)
                        late_loads(tcb)
                        if tcb in (0, 2, 4):  # K proj t-blocks 1..3 (group 0/1)
                            k_proj_tb(0, tcb // 2 + 1)
                    exb = work.tile([128, 2, 2, 512], f8, tag="exb", bufs=4)
                    tiles = []
                    for u in range(2):
                        tcc = 2 * tcb + u
                        ps2 = psS.tile([128, 2, 512], f32, tag="sc", name="ps2")
                        tiles.append((tcc, ps2))
                    # row-tiled pair: head 2c on PE rows 0-63, head 2c+1 on
                    # rows 64-127 -> the j=0/j=1 matmuls run concurrently
                    for tcc, ps2 in tiles:
                        nc.tensor.matmul(
                            ps2[:, 0, :],
                            lhsT=ktz[0:64, 2 * g, tcc * 128:(tcc + 1) * 128],
                            rhs=qt[0:64, c, :],
                            start=True,
                            stop=True,
                        )
                        nc.tensor.matmul(
                            ps2[:, 1, :],
                            lhsT=ktz[64:128, 2 * g + 1, tcc * 128:(tcc + 1) * 128],
                            rhs=qt[64:128, c, :],
                            start=True,
                            stop=True,
                        )
                    for u, (tcc, ps2) in enumerate(tiles):
                        nc.scalar.activation(exb[:, u, :, :], ps2[:], Exp)
                    exbs[(c, tcb)] = exb
                # tail of previous pair in fixed group slots
                if c > 0:
                    b = c - 1
                    if tcb == 0:
                        pv_mm(b, 6)
                    elif tcb == 1:
                        pv_mm(b, 7)
                        pv_evict(b, drain=(c == 8))
                    elif tcb == 2:
                        recips(b, drain=(c == 8))
                    elif tcb == 3:
                        norm_head(b, 0)
                    elif tcb == 4:
                        norm_head(b, 1)
                    elif tcb in (5, 6, 7) and c in (2, 4, 6):
                        for i in range((tcb - 5) * 3, min((tcb - 4) * 3, 8)):
                            post_chunk2(c - 2, i)
                if c == 7:
                    if tcb < 4:  # residual pre-add (pacc has chunks 0..5)
                        xres_preadd(2 * tcb)
                        xres_preadd(2 * tcb + 1)
                    elif tcb in (5, 6, 7):  # chunk 6 posts under the last exps
                        for i in range((tcb - 5) * 3, min((tcb - 4) * 3, 8)):
                            post_single(6, i)
                if c == 8 and tcb in (5, 6, 7):  # drain: chunk 7 + output
                    for i in range((tcb - 5) * 3, min((tcb - 4) * 3, 8)):
                        post_single(7, i)
                if c < 6 and tcb == (6 if c == 0 else 5):
                    q_proj(c + 2)
                # trailing PV for this pair (2 tcb behind)
                if c < 8 and tcb >= 2:
                    pv_mm(c, tcb - 2)

        def ores_out(i):
            sc, dc = i // 2, i % 2
            dsl = slice(dc * 512, (dc + 1) * 512)
            ores = work.tile([128, 512], f32, tag="or")
            eng = nc.vector if i % 2 == 0 else nc.gpsimd
            eng.tensor_add(ores[:], pacc[:, sc, dsl], xres[:, sc, dsl])
            dq = nc.sync if i % 2 == 0 else nc.gpsimd
            dq.dma_start(out=out_d[sc, :, dsl], in_=ores[:])

        k_proj_tb(0, 0)
        q_proj(0)
        pair_blocks(0)  # k_proj_tb(0, 1..3) run inside pair 0's slots
        q_proj(1)
        pair_blocks(1)
        for tb in range(4):
            k_proj_tb(1, tb)
        for c in range(2, 8):
            pair_blocks(c)
        pair_blocks(8)  # drain: tail of pair 7, posts for chunks 6,7, output


    nc.compile()
    return nc


def get_program():
    if "nc" not in _prog_cache:
        _prog_cache["nc"] = _build_program()
    return _prog_cache["nc"]


def _chunk128(a):
    n = a.shape[1]
    return np.ascontiguousarray(a.reshape(8, 128, n).transpose(1, 0, 2))


def make_in_maps(X, Wq, Wk, Wv, Wpost):
    X = np.asarray(X, dtype=np.float32)
    wq_p = _chunk128(np.asarray(Wq, dtype=np.float32)).astype(FP8)
    wk_p = _chunk128(np.asarray(Wk, dtype=np.float32)).astype(FP8)
    wv_p = _chunk128(np.asarray(Wv, dtype=np.float32)).astype(FP8)
    wp_p = _chunk128(np.asarray(Wpost, dtype=np.float32)).astype(FP8)

    xt_b = []
    for b in range(B):
        xt_b.append(_chunk128(np.ascontiguousarray(X[b].T)).astype(FP8))

    in_maps = []
    for core in range(NCORES):
        b = core // CORES_PER_BATCH
        q0 = (core % CORES_PER_BATCH) * SLOC
        xt = xt_b[b]
        xres = np.ascontiguousarray(
            X[b, q0:q0 + SLOC].reshape(4, 128, D).transpose(1, 0, 2)
        )
        in_maps.append(
            {
                "XT": xt,
                "XTQ": np.ascontiguousarray(xt[:, :, q0:q0 + SLOC]),
                "XRES": xres,
                "WQ": wq_p,
                "WK": wk_p,
                "WV": wv_p,
                "WP": wp_p,
            }
        )
    return in_maps


def assemble_output(results):
    out = np.empty((B, S, D), dtype=np.float32)
    for core, r in enumerate(results):
        b = core // CORES_PER_BATCH
        q0 = (core % CORES_PER_BATCH) * SLOC
        out[b, q0:q0 + SLOC] = np.asarray(r["OUT"]).reshape(SLOC, D)
    return out


def kernel(X, Wq, Wk, Wv, Wpost, _trace=False):
    from concourse.bass_utils import run_bass_kernel_spmd

    nc = get_program()
    in_maps = make_in_maps(X, Wq, Wk, Wv, Wpost)
    res = run_bass_kernel_spmd(nc, in_maps, core_ids=list(range(NCORES)), trace=_trace)
    out = assemble_output(res.results)
    if _trace:
        return out, res
    return out


# revision 37
# speedup vs baseline: 1.0809x; 1.0809x over previous
"""GroupedQueryAttentionLayer on 8 trn2 NeuronCores (Bass/Tile, SPMD).

Sharding: data-parallel over query rows; no collectives. Core i handles
batch b = i//4, query rows q0 = (i%4)*512 .. +512. Each core recomputes
its batch's K/V projection (cheap vs. attention); outputs are disjoint
row-slices of the final [2, 2048, 1024].

Host-prepared per-core layouts (transposes/casts are part of sharding):
  XT   [128, 8, 2048] fp8e4 : X[b].T k-chunked (XT[p,c,t] = X[b,t,c*128+p])
  XTQ  [128, 8,  512] fp8e4 : XT columns q0..q0+512 (this core's queries)
  XRES [128, 4, 1024] bf16  : residual rows
  WQ/WP [128, 8, 1024] fp8e4, WK/WV [128, 8, 256] fp8e4 (k-chunked weights)
Output OUT [4, 128, 1024] f32, OUT[sc, p, :] = row q0 + sc*128 + p.

Kernel: softmax kept with t on PSUM partitions.
- Projections (Q/K/V/post) run as fp8 DoubleRow matmuls: each instruction
  contracts two 128-row k-chunks (lhsT/rhs laid out [128, 2, m]) at 2 fp8
  rows/clk, halving PE time vs bf16.
- Scores stay bf16 for accuracy but are ROW-TILED: per head pair, head 2c
  occupies PE rows 0-63 (K=64 via tile_position (0,0)) and head 2c+1 rows
  64-127 ((64,0)); the two matmuls execute concurrently on the array
  halves, halving score PE time. ktz keeps K_g at both parities so the
  64-row slices line up with qt's head layout.
- V carries a ones column so the PV DoubleRow matmul emits the softmax
  denominator as row 64. exp on ScalarE from 2-bank PSUM blocks straight
  to fp8 (no max subtraction: |scores| <= ~4 by construction).
- Head pairs are software-pipelined: scores/exp of pair c run while pair
  c-1's PV-finish, SBUF-evict, partition-spread reciprocal,
  broadcast-matmul normalize, and post-projection fill fixed slots. The
  post output accumulates in SBUF; the residual-add streams out.
"""

import math

import numpy as np
import ml_dtypes

BF16 = ml_dtypes.bfloat16
FP8 = ml_dtypes.float8_e4m3  # TRN float8e4 (e4m3, max 240)

B, S, D = 2, 2048, 1024
HEADS, GROUPS, E = 16, 4, 64
HPG = HEADS // GROUPS
NCORES = 8
CORES_PER_BATCH = NCORES // B
SLOC = B * S // NCORES
SCALE = 1.0 / math.sqrt(E)

_prog_cache = {}


def _build_program():
    from contextlib import ExitStack

    import concourse.bacc as bacc
    import concourse.tile as tile
    from concourse import mybir

    f32 = mybir.dt.float32
    b16 = mybir.dt.bfloat16
    f8 = mybir.dt.float8e4
    Exp = mybir.ActivationFunctionType.Exp
    DR = mybir.MatmulPerfMode.DoubleRow

    nc = bacc.Bacc("TRN2", target_bir_lowering=False)

    xt_d = nc.dram_tensor("XT", [128, 8, S], f8, kind="ExternalInput")
    xtq_d = nc.dram_tensor("XTQ", [128, 8, SLOC], f8, kind="ExternalInput")
    xres_d = nc.dram_tensor("XRES", [128, 4, D], b16, kind="ExternalInput")
    wq_d = nc.dram_tensor("WQ", [128, 8, 1024], f8, kind="ExternalInput")
    wk_d = nc.dram_tensor("WK", [128, 8, 256], f8, kind="ExternalInput")
    wv_d = nc.dram_tensor("WV", [128, 8, 256], f8, kind="ExternalInput")
    wp_d = nc.dram_tensor("WP", [128, 8, 1024], f8, kind="ExternalInput")
    out_d = nc.dram_tensor("OUT", [4, 128, D], f32, kind="ExternalOutput")

    with tile.TileContext(nc) as tc, ExitStack() as ctx:
        consts = ctx.enter_context(tc.tile_pool(name="consts", bufs=1))
        work = ctx.enter_context(tc.tile_pool(name="work", bufs=2))
        # PSUM (8 banks): scores 2x2 + pv 2x1 + pp(proj/post/bcast) 2x1
        psA = ctx.enter_context(tc.tile_pool(name="psA", bufs=2, space="PSUM"))
        psS = ctx.enter_context(tc.tile_pool(name="psS", bufs=2, space="PSUM"))
        psV = ctx.enter_context(tc.tile_pool(name="psV", bufs=1, space="PSUM"))

        xt = consts.tile([128, 8, S], f8)
        xtq = consts.tile([128, 8, SLOC], f8)
        xres = consts.tile([128, 4, D], b16)
        wq = consts.tile([128, 8, 1024], f8)
        wk = consts.tile([128, 8, 256], f8)
        wv = consts.tile([128, 8, 256], f8)
        wp = consts.tile([128, 8, 1024], f8)
        ktz = consts.tile([128, 8, S], b16)  # slot 2g+j holds K_g at parity j rows
        # last dim padded 65->68 so the t-chunk-pair stride (4*68=272B) meets
        # the DoubleRow ldweights step%16==0 restriction
        vpr = consts.tile([128, 16, 4, E + 4], f8)
        qt = consts.tile([128, 8, SLOC], b16)
        atn = consts.tile([128, 8, SLOC], f8)
        pacc = consts.tile([128, 4, D], f32)  # streamed post accumulation
        e64 = consts.tile([128, 128], b16)  # row 64 = 1, else 0 (K=128 bcast)
        rbe = consts.tile([128, 512], b16)  # recip staging, rows != 64 stay 0
        rbo = consts.tile([128, 512], b16)
        warm = consts.tile([128, 8], f32)

        nc.vector.memset(e64[:], 0.0)
        nc.vector.memset(e64[64:65, :], 1.0)
        nc.vector.memset(rbe[:], 0.0)
        nc.vector.memset(rbo[:], 0.0)
        nc.vector.memset(vpr[:, :, :, E:E + 1], 1.0)  # ones column only
        nc.vector.memset(warm[:], 0.0)
        nc.scalar.activation(warm[:], warm[:], Exp)  # exp table preload

        # All input DMAs share one ~358GB/s HBM stream: issue strictly in
        # first-use order. Late-use bulk (wq tail / wp / xres) is triggered
        # from inside pair 0 so it can't jump ahead of the xt chunks.
        nc.sync.dma_start(out=wk[:], in_=wk_d[:])
        nc.sync.dma_start(out=wv[:], in_=wv_d[:])
        nc.sync.dma_start(out=xt[:, :, 0:512], in_=xt_d[:, :, 0:512])
        nc.sync.dma_start(out=wq[:, :, 0:256], in_=wq_d[:, :, 0:256])
        nc.sync.dma_start(out=xtq[:], in_=xtq_d[:])
        for t4 in range(1, 4):
            sl = slice(t4 * 512, (t4 + 1) * 512)
            nc.sync.dma_start(out=xt[:, :, sl], in_=xt_d[:, :, sl])

        def late_loads(tcb):
            # triggered from the Scalar strict-FIFO queue (behind pair-0's
            # exps) so the transfers stay behind the critical loads in the
            # shared HBM stream; gpsimd triggers get hoisted to t=0 (its 8
            # Q7 queues run independently)
            if tcb == 0:
                nc.scalar.dma_start(out=wq[:, :, 256:1024], in_=wq_d[:, :, 256:1024])
            elif tcb == 2:
                nc.scalar.dma_start(out=wp[:], in_=wp_d[:])
            elif tcb == 4:
                nc.scalar.dma_start(out=xres[:], in_=xres_d[:])

        def k_proj_tb(ec, tb):
            ps = psA.tile([128, 512], f32, tag="pp")
            for ki in range(4):
                nc.tensor.matmul(
                    ps[:],
                    lhsT=wk[:, 2 * ki:2 * ki + 2, ec * 128:(ec + 1) * 128],
                    rhs=xt[:, 2 * ki:2 * ki + 2, tb * 512:(tb + 1) * 512],
                    start=(ki == 0),
                    stop=(ki == 3),
                    perf_mode=DR,
                )
            sl = slice(tb * 512, (tb + 1) * 512)
            ga, gb = 2 * ec, 2 * ec + 1
            nc.vector.tensor_copy(ktz[0:64, 2 * ga, sl], ps[0:64, :])
            nc.vector.tensor_copy(ktz[64:128, 2 * gb + 1, sl], ps[64:128, :])
            nc.gpsimd.dma_start(out=ktz[64:128, 2 * ga + 1, sl], in_=ktz[0:64, 2 * ga, sl])
            nc.gpsimd.dma_start(out=ktz[0:64, 2 * gb, sl], in_=ktz[64:128, 2 * gb + 1, sl])

        def q_proj(hc):
            ps = psA.tile([128, 512], f32, tag="pp")
            for ki in range(4):
                nc.tensor.matmul(
                    ps[:],
                    lhsT=wq[:, 2 * ki:2 * ki + 2, hc * 128:(hc + 1) * 128],
                    rhs=xtq[:, 2 * ki:2 * ki + 2, :],
                    start=(ki == 0),
                    stop=(ki == 3),
                    perf_mode=DR,
                )
            nc.vector.tensor_scalar_mul(qt[:, hc, :], ps, SCALE)

        def v_proj_tcc(tcc):
            ps = psA.tile([128, 256], f32, tag="pp")
            for ki in range(4):
                nc.tensor.matmul(
                    ps[:],
                    lhsT=xt[:, 2 * ki:2 * ki + 2, tcc * 128:(tcc + 1) * 128],
                    rhs=wv[:, 2 * ki:2 * ki + 2, :],
                    start=(ki == 0),
                    stop=(ki == 3),
                    perf_mode=DR,
                )
            nc.vector.tensor_copy(
                vpr[:, tcc, :, 0:E], ps.rearrange("p (g e) -> p g e", g=4)
            )

        state = {}  # live psV tiles per pair: c -> (pve, pvo)

        def pv_mm(c, tp):
            """PV DoubleRow step tp (t-chunks 2tp, 2tp+1) for both heads."""
            g = c // 2
            if tp == 0:
                state[c] = (
                    psV.tile([E + 1, 512], f32, tag="pve", name="pve"),
                    psV.tile([E + 1, 512], f32, tag="pvo", name="pvo"),
                )
            pve, pvo = state[c]
            exb = exbs.pop((c, tp))
            for j, pv in ((0, pve), (1, pvo)):
                nc.tensor.matmul(
                    pv[:],
                    lhsT=vpr[:, 2 * tp:2 * tp + 2, g, 0:E + 1],
                    rhs=exb[:, :, j, :],
                    start=(tp == 0),
                    stop=(tp == 7),
                    perf_mode=DR,
                )

        aun = {}

        def pv_evict(c, drain=False):
            """Copy A' to SBUF right after PV stop so the PSUM slots free
            early; the normalize chain then runs from SBUF. In the drain
            (no exp left) ScalarE is idle: use it for the casts."""
            pve, pvo = state.pop(c)
            te = work.tile([65, 512], b16, tag="aune", name="aune")
            to = work.tile([65, 512], b16, tag="auno", name="auno")
            if drain:
                nc.scalar.copy(te[:], pve[:])
                nc.scalar.copy(to[:], pvo[:])
            else:
                nc.vector.tensor_copy(te[:], pve[:])
                nc.vector.tensor_copy(to[:], pvo[:])
            aun[c] = (te, to)

        def recips(c, drain=False):
            te, to = aun[c]
            for t, rb in ((te, rbe), (to, rbo)):
                # spread the 512 denominators over 64 partitions so the DVE
                # reciprocal runs at 8 elements/lane instead of 512
                dsp = work.tile([64, 8], b16, tag="dsp")
                nc.gpsimd.dma_start(
                    out=dsp[:, None, :],
                    in_=t[64:65, :].rearrange("p (a b) -> p a b", a=64),
                )
                rsp = work.tile([64, 8], b16, tag="rsp")
                with nc.allow_low_precision(reason="bf16 softmax recip"):
                    nc.vector.reciprocal(rsp[:], dsp[:])
                nc.gpsimd.dma_start(
                    out=rb[64:65, :].rearrange("p (a b) -> p a b", a=64),
                    in_=rsp[:, None, :],
                )

        def norm_head(c, j):
            te, to = aun[c]
            t, rb = (te, rbe) if j == 0 else (to, rbo)
            bc = psA.tile([128, 512], f32, tag="pp")
            nc.tensor.matmul(bc[:], lhsT=e64[:], rhs=rb[:], start=True, stop=True)
            bcs = work.tile([64, 512], b16, tag="bc")
            nc.vector.tensor_copy(bcs[:], bc[0:64, :])
            if j == 0:
                nc.vector.tensor_mul(atn[0:64, c, :], t[0:64, :], bcs[:])
            else:
                so = work.tile([64, 512], f8, tag="so")
                nc.vector.tensor_mul(so[:], t[0:64, :], bcs[:])
                nc.gpsimd.dma_start(out=atn[64:128, c, :], in_=so[:])
                aun.pop(c)

        def post_chunk2(c0, i):
            """One DoubleRow matmul: he-chunks c0,c0+1 into one PSUM tile."""
            sc, dc = i // 2, i % 2
            pp = psA.tile([128, 512], f32, tag="pp")
            nc.tensor.matmul(
                pp[:],
                lhsT=atn[:, c0:c0 + 2, sc * 128:(sc + 1) * 128],
                rhs=wp[:, c0:c0 + 2, dc * 512:(dc + 1) * 512],
                start=True,
                stop=True,
                perf_mode=DR,
            )
            dsl = slice(dc * 512, (dc + 1) * 512)
            if c0 == 0:
                nc.vector.tensor_copy(pacc[:, sc, dsl], pp[:])
            else:
                nc.vector.tensor_add(pacc[:, sc, dsl], pacc[:, sc, dsl], pp[:])

        def xres_preadd(i):
            """Fold the residual into pacc while exp still streams (pair 7)."""
            sc, dc = i // 2, i % 2
            dsl = slice(dc * 512, (dc + 1) * 512)
            nc.vector.tensor_add(pacc[:, sc, dsl], pacc[:, sc, dsl], xres[:, sc, dsl])

        def post_single(cx, i):
            """Single-chunk post matmul (K=128, no DoubleRow). For chunk 6 it
            accumulates into pacc during pair 7; for chunk 7 (the drain) it
            fuses the final add and streams the output out."""
            sc, dc = i // 2, i % 2
            pp = psA.tile([128, 512], f32, tag="pp")
            nc.tensor.matmul(
                pp[:],
                lhsT=atn[:, cx, sc * 128:(sc + 1) * 128],
                rhs=wp[:, cx, dc * 512:(dc + 1) * 512],
                start=True,
                stop=True,
            )
            dsl = slice(dc * 512, (dc + 1) * 512)
            if cx == 6:
                nc.vector.tensor_add(pacc[:, sc, dsl], pacc[:, sc, dsl], pp[:])
            else:
                ores = work.tile([128, 512], f32, tag="or")
                nc.vector.tensor_add(ores[:], pacc[:, sc, dsl], pp[:])
                dq = nc.sync if i % 2 == 0 else nc.gpsimd
                dq.dma_start(out=out_d[sc, :, dsl], in_=ores[:])

        exbs = {}

        def pair_blocks(c):
            """One pipeline step: scores/exp for pair c (row-tiled 64-row
            matmuls, heads on PE halves run concurrently), trailing PV for c,
            and the tail (PV-finish, normalize, post) of pair c-1."""
            g = c // 2
            for tcb in range(8):
                if c < 8:
                    if c == 0:
                        v_proj_tcc(2 * tcb)
                        v_proj_tcc(2 * tcb + 1)
                        late_loads(tcb)
                        if tcb in (0, 2, 4):  # K proj t-blocks 1..3 (group 0/1)
                            k_proj_tb(0, tcb // 2 + 1)
                    exb = work.tile([128, 2, 2, 512], f8, tag="exb", bufs=4)
                    tiles = []
                    for u in range(2):
                        tcc = 2 * tcb + u
                        ps2 = psS.tile([128, 2, 512], f32, tag="sc", name="ps2")
                        tiles.append((tcc, ps2))
                    # row-tiled pair: head 2c on PE rows 0-63, head 2c+1 on
                    # rows 64-127 -> the j=0/j=1 matmuls run concurrently
                    for tcc, ps2 in tiles:
                        nc.tensor.matmul(
                            ps2[:, 0, :],
                            lhsT=ktz[0:64, 2 * g, tcc * 128:(tcc + 1) * 128],
                            rhs=qt[0:64, c, :],
                            start=True,
                            stop=True,
                        )
                        nc.tensor.matmul(
                            ps2[:, 1, :],
                            lhsT=ktz[64:128, 2 * g + 1, tcc * 128:(tcc + 1) * 128],
                            rhs=qt[64:128, c, :],
                            start=True,
                            stop=True,
                        )
                    for u, (tcc, ps2) in enumerate(tiles):
                        nc.scalar.activation(exb[:, u, :, :], ps2[:], Exp)
                    exbs[(c, tcb)] = exb
                # tail of previous pair in fixed group slots
                if c > 0:
                    b = c - 1
                    if tcb == 0:
                        pv_mm(b, 6)
                    elif tcb == 1:
                        pv_mm(b, 7)
                        pv_evict(b, drain=(c == 8))
                    elif tcb == 2:
                        recips(b, drain=(c == 8))
                    elif tcb == 3:
                        norm_head(b, 0)
                    elif tcb == 4:
                        norm_head(b, 1)
                    elif tcb in (5, 6, 7) and c in (2, 4, 6):
                        for i in range((tcb - 5) * 3, min((tcb - 4) * 3, 8)):
                            post_chunk2(c - 2, i)
                if c == 7:
                    if tcb < 4:  # residual pre-add (pacc has chunks 0..5)
                        xres_preadd(2 * tcb)
                        xres_preadd(2 * tcb + 1)
                    elif tcb in (5, 6, 7):  # chunk 6 posts under the last exps
                        for i in range((tcb - 5) * 3, min((tcb - 4) * 3, 8)):
                            post_single(6, i)
                if c == 8 and tcb in (5, 6, 7):  # drain: chunk 7 + output
                    for i in range((tcb - 5) * 3, min((tcb - 4) * 3, 8)):
                        post_single(7, i)
                if c < 6 and tcb == (6 if c == 0 else 5):
                    q_proj(c + 2)
                # trailing PV for this pair (2 tcb behind)
                if c < 8 and tcb >= 2:
                    pv_mm(c, tcb - 2)

        def ores_out(i):
            sc, dc = i // 2, i % 2
            dsl = slice(dc * 512, (dc + 1) * 512)
            ores = work.tile([128, 512], f32, tag="or")
            eng = nc.vector if i % 2 == 0 else nc.gpsimd
            eng.tensor_add(ores[:], pacc[:, sc, dsl], xres[:, sc, dsl])
            dq = nc.sync if i % 2 == 0 else nc.gpsimd
            dq.dma_start(out=out_d[sc, :, dsl], in_=ores[:])

        k_proj_tb(0, 0)
        q_proj(0)
        pair_blocks(0)  # k_proj_tb(0, 1..3) run inside pair 0's slots
        q_proj(1)
        pair_blocks(1)
        for tb in range(4):
            k_proj_tb(1, tb)
        for c in range(2, 8):
            pair_blocks(c)
        pair_blocks(8)  # drain: tail of pair 7, posts for chunks 6,7, output


    nc.compile()
    return nc


def get_program():
    if "nc" not in _prog_cache:
        _prog_cache["nc"] = _build_program()
    return _prog_cache["nc"]


def _chunk128(a):
    n = a.shape[1]
    return np.ascontiguousarray(a.reshape(8, 128, n).transpose(1, 0, 2))


def make_in_maps(X, Wq, Wk, Wv, Wpost):
    X = np.asarray(X, dtype=np.float32)
    wq_p = _chunk128(np.asarray(Wq, dtype=np.float32)).astype(FP8)
    wk_p = _chunk128(np.asarray(Wk, dtype=np.float32)).astype(FP8)
    wv_p = _chunk128(np.asarray(Wv, dtype=np.float32)).astype(FP8)
    wp_p = _chunk128(np.asarray(Wpost, dtype=np.float32)).astype(FP8)

    xt_b = []
    for b in range(B):
        xt_b.append(_chunk128(np.ascontiguousarray(X[b].T)).astype(FP8))

    in_maps = []
    for core in range(NCORES):
        b = core // CORES_PER_BATCH
        q0 = (core % CORES_PER_BATCH) * SLOC
        xt = xt_b[b]
        xres = np.ascontiguousarray(
            X[b, q0:q0 + SLOC].reshape(4, 128, D).transpose(1, 0, 2)
        ).astype(BF16)
        in_maps.append(
            {
                "XT": xt,
                "XTQ": np.ascontiguousarray(xt[:, :, q0:q0 + SLOC]),
                "XRES": xres,
                "WQ": wq_p,
                "WK": wk_p,
                "WV": wv_p,
                "WP": wp_p,
            }
        )
    return in_maps


def assemble_output(results):
    out = np.empty((B, S, D), dtype=np.float32)
    for core, r in enumerate(results):
        b = core // CORES_PER_BATCH
        q0 = (core % CORES_PER_BATCH) * SLOC
        out[b, q0:q0 + SLOC] = np.asarray(r["OUT"]).reshape(SLOC, D)
    return out


def kernel(X, Wq, Wk, Wv, Wpost, _trace=False):
    from concourse.bass_utils import run_bass_kernel_spmd

    nc = get_program()
    in_maps = make_in_maps(X, Wq, Wk, Wv, Wpost)
    res = run_bass_kernel_spmd(nc, in_maps, core_ids=list(range(NCORES)), trace=_trace)
    out = assemble_output(res.results)
    if _trace:
        return out, res
    return out


# revision 39
# speedup vs baseline: 1.0820x; 1.0010x over previous
"""GroupedQueryAttentionLayer on 8 trn2 NeuronCores (Bass/Tile, SPMD).

Sharding: data-parallel over query rows; no collectives. Core i handles
batch b = i//4, query rows q0 = (i%4)*512 .. +512. Each core recomputes
its batch's K/V projection (cheap vs. attention); outputs are disjoint
row-slices of the final [2, 2048, 1024].

Host-prepared per-core layouts (transposes/casts are part of sharding):
  XT   [128, 8, 2048] fp8e4 : X[b].T k-chunked (XT[p,c,t] = X[b,t,c*128+p])
  XTQ  [128, 8,  512] fp8e4 : XT columns q0..q0+512 (this core's queries)
  XRES [128, 4, 1024] bf16  : residual rows
  WQ/WP [128, 8, 1024] fp8e4, WK/WV [128, 8, 256] fp8e4 (k-chunked weights)
Output OUT [4, 128, 1024] f32, OUT[sc, p, :] = row q0 + sc*128 + p.

Kernel: softmax kept with t on PSUM partitions.
- Projections (Q/K/V/post) run as fp8 DoubleRow matmuls: each instruction
  contracts two 128-row k-chunks (lhsT/rhs laid out [128, 2, m]) at 2 fp8
  rows/clk, halving PE time vs bf16.
- Scores stay bf16 for accuracy but are ROW-TILED: per head pair, head 2c
  occupies PE rows 0-63 (K=64 via tile_position (0,0)) and head 2c+1 rows
  64-127 ((64,0)); the two matmuls execute concurrently on the array
  halves, halving score PE time. ktz keeps K_g at both parities so the
  64-row slices line up with qt's head layout.
- V carries a ones column so the PV DoubleRow matmul emits the softmax
  denominator as row 64. exp on ScalarE from 2-bank PSUM blocks straight
  to fp8 (no max subtraction: |scores| <= ~4 by construction).
- Head pairs are software-pipelined: scores/exp of pair c run while pair
  c-1's PV-finish, SBUF-evict, partition-spread reciprocal,
  broadcast-matmul normalize, and post-projection fill fixed slots. The
  post output accumulates in SBUF; the residual-add streams out.
"""

import math

import numpy as np
import ml_dtypes

BF16 = ml_dtypes.bfloat16
FP8 = ml_dtypes.float8_e4m3  # TRN float8e4 (e4m3, max 240)

B, S, D = 2, 2048, 1024
HEADS, GROUPS, E = 16, 4, 64
HPG = HEADS // GROUPS
NCORES = 8
CORES_PER_BATCH = NCORES // B
SLOC = B * S // NCORES
SCALE = 1.0 / math.sqrt(E)

_prog_cache = {}


def _build_program():
    from contextlib import ExitStack

    import concourse.bacc as bacc
    import concourse.tile as tile
    from concourse import mybir

    f32 = mybir.dt.float32
    b16 = mybir.dt.bfloat16
    f8 = mybir.dt.float8e4
    Exp = mybir.ActivationFunctionType.Exp
    DR = mybir.MatmulPerfMode.DoubleRow

    nc = bacc.Bacc("TRN2", target_bir_lowering=False)

    xt_d = nc.dram_tensor("XT", [128, 8, S], f8, kind="ExternalInput")
    xtq_d = nc.dram_tensor("XTQ", [128, 8, SLOC], f8, kind="ExternalInput")
    xres_d = nc.dram_tensor("XRES", [128, 4, D], b16, kind="ExternalInput")
    wq_d = nc.dram_tensor("WQ", [128, 8, 1024], f8, kind="ExternalInput")
    wk_d = nc.dram_tensor("WK", [128, 8, 256], f8, kind="ExternalInput")
    wv_d = nc.dram_tensor("WV", [128, 8, 256], f8, kind="ExternalInput")
    wp_d = nc.dram_tensor("WP", [128, 8, 1024], f8, kind="ExternalInput")
    out_d = nc.dram_tensor("OUT", [4, 128, D], f32, kind="ExternalOutput")

    with tile.TileContext(nc) as tc, ExitStack() as ctx:
        consts = ctx.enter_context(tc.tile_pool(name="consts", bufs=1))
        work = ctx.enter_context(tc.tile_pool(name="work", bufs=2))
        # PSUM (8 banks): scores 2x2 + pv 2x1 + pp(proj/post/bcast) 2x1
        psA = ctx.enter_context(tc.tile_pool(name="psA", bufs=2, space="PSUM"))
        psS = ctx.enter_context(tc.tile_pool(name="psS", bufs=2, space="PSUM"))
        psV = ctx.enter_context(tc.tile_pool(name="psV", bufs=1, space="PSUM"))

        xt = consts.tile([128, 8, S], f8)
        xtq = consts.tile([128, 8, SLOC], f8)
        xres = consts.tile([128, 4, D], b16)
        wq = consts.tile([128, 8, 1024], f8)
        wk = consts.tile([128, 8, 256], f8)
        wv = consts.tile([128, 8, 256], f8)
        wp = consts.tile([128, 8, 1024], f8)
        ktz = consts.tile([128, 8, S], b16)  # slot 2g+j holds K_g at parity j rows
        # last dim padded 65->68 so the t-chunk-pair stride (4*68=272B) meets
        # the DoubleRow ldweights step%16==0 restriction
        vpr = consts.tile([128, 16, 4, E + 4], f8)
        qt = consts.tile([128, 8, SLOC], b16)
        atn = consts.tile([128, 8, SLOC], f8)
        pacc = consts.tile([128, 4, D], f32)  # streamed post accumulation
        e64 = consts.tile([128, 128], b16)  # row 64 = 1, else 0 (K=128 bcast)
        rbe = consts.tile([128, 512], b16)  # recip staging, rows != 64 stay 0
        rbo = consts.tile([128, 512], b16)
        warm = consts.tile([128, 8], f32)

        nc.vector.memset(e64[:], 0.0)
        nc.vector.memset(e64[64:65, :], 1.0)
        nc.vector.memset(rbe[:], 0.0)
        nc.vector.memset(rbo[:], 0.0)
        nc.vector.memset(vpr[:, :, :, E:E + 1], 1.0)  # ones column only
        nc.vector.memset(warm[:], 0.0)
        nc.scalar.activation(warm[:], warm[:], Exp)  # exp table preload

        # All input DMAs share one ~358GB/s HBM stream: issue strictly in
        # first-use order. Late-use bulk (wq tail / wp / xres) is triggered
        # from inside pair 0 so it can't jump ahead of the xt chunks.
        nc.sync.dma_start(out=wk[:], in_=wk_d[:])
        nc.sync.dma_start(out=wv[:], in_=wv_d[:])
        nc.sync.dma_start(out=xt[:, :, 0:512], in_=xt_d[:, :, 0:512])
        nc.sync.dma_start(out=wq[:, :, 0:256], in_=wq_d[:, :, 0:256])
        nc.sync.dma_start(out=xtq[:], in_=xtq_d[:])
        for t4 in range(1, 4):
            sl = slice(t4 * 512, (t4 + 1) * 512)
            nc.sync.dma_start(out=xt[:, :, sl], in_=xt_d[:, :, sl])

        def late_loads(tcb):
            # triggered from the Scalar strict-FIFO queue (behind pair-0's
            # exps) so the transfers stay behind the critical loads in the
            # shared HBM stream; gpsimd triggers get hoisted to t=0 (its 8
            # Q7 queues run independently)
            if tcb == 0:
                nc.scalar.dma_start(out=wq[:, :, 256:1024], in_=wq_d[:, :, 256:1024])
            elif tcb == 2:
                nc.scalar.dma_start(out=wp[:], in_=wp_d[:])
            elif tcb == 4:
                nc.scalar.dma_start(out=xres[:], in_=xres_d[:])

        def k_proj_tb(ec, tb):
            ps = psA.tile([128, 512], f32, tag="pp")
            for ki in range(4):
                nc.tensor.matmul(
                    ps[:],
                    lhsT=wk[:, 2 * ki:2 * ki + 2, ec * 128:(ec + 1) * 128],
                    rhs=xt[:, 2 * ki:2 * ki + 2, tb * 512:(tb + 1) * 512],
                    start=(ki == 0),
                    stop=(ki == 3),
                    perf_mode=DR,
                )
            sl = slice(tb * 512, (tb + 1) * 512)
            ga, gb = 2 * ec, 2 * ec + 1
            nc.vector.tensor_copy(ktz[0:64, 2 * ga, sl], ps[0:64, :])
            nc.vector.tensor_copy(ktz[64:128, 2 * gb + 1, sl], ps[64:128, :])
            nc.gpsimd.dma_start(out=ktz[64:128, 2 * ga + 1, sl], in_=ktz[0:64, 2 * ga, sl])
            nc.gpsimd.dma_start(out=ktz[0:64, 2 * gb, sl], in_=ktz[64:128, 2 * gb + 1, sl])

        def q_proj(hc):
            ps = psA.tile([128, 512], f32, tag="pp")
            for ki in range(4):
                nc.tensor.matmul(
                    ps[:],
                    lhsT=wq[:, 2 * ki:2 * ki + 2, hc * 128:(hc + 1) * 128],
                    rhs=xtq[:, 2 * ki:2 * ki + 2, :],
                    start=(ki == 0),
                    stop=(ki == 3),
                    perf_mode=DR,
                )
            nc.vector.tensor_scalar_mul(qt[:, hc, :], ps, SCALE)

        def v_proj_tcc(tcc):
            ps = psA.tile([128, 256], f32, tag="pp")
            for ki in range(4):
                nc.tensor.matmul(
                    ps[:],
                    lhsT=xt[:, 2 * ki:2 * ki + 2, tcc * 128:(tcc + 1) * 128],
                    rhs=wv[:, 2 * ki:2 * ki + 2, :],
                    start=(ki == 0),
                    stop=(ki == 3),
                    perf_mode=DR,
                )
            nc.vector.tensor_copy(
                vpr[:, tcc, :, 0:E], ps.rearrange("p (g e) -> p g e", g=4)
            )

        state = {}  # live psV tiles per pair: c -> (pve, pvo)

        def pv_mm(c, tp):
            """PV DoubleRow step tp (t-chunks 2tp, 2tp+1) for both heads."""
            g = c // 2
            if tp == 0:
                state[c] = (
                    psV.tile([E + 1, 512], f32, tag="pve", name="pve"),
                    psV.tile([E + 1, 512], f32, tag="pvo", name="pvo"),
                )
            pve, pvo = state[c]
            exb = exbs.pop((c, tp))
            for j, pv in ((0, pve), (1, pvo)):
                nc.tensor.matmul(
                    pv[:],
                    lhsT=vpr[:, 2 * tp:2 * tp + 2, g, 0:E + 1],
                    rhs=exb[:, :, j, :],
                    start=(tp == 0),
                    stop=(tp == 7),
                    perf_mode=DR,
                )

        aun = {}

        def pv_evict(c, drain=False):
            """Copy A' to SBUF right after PV stop so the PSUM slots free
            early; the normalize chain then runs from SBUF. In the drain
            (no exp left) ScalarE is idle: use it for the casts."""
            pve, pvo = state.pop(c)
            te = work.tile([65, 512], b16, tag="aune", name="aune")
            to = work.tile([65, 512], b16, tag="auno", name="auno")
            if drain:
                nc.scalar.copy(te[:], pve[:])
                nc.scalar.copy(to[:], pvo[:])
            else:
                nc.vector.tensor_copy(te[:], pve[:])
                nc.vector.tensor_copy(to[:], pvo[:])
            aun[c] = (te, to)

        def recips(c, drain=False):
            te, to = aun[c]
            for t, rb in ((te, rbe), (to, rbo)):
                # spread the 512 denominators over 64 partitions so the DVE
                # reciprocal runs at 8 elements/lane instead of 512
                dsp = work.tile([64, 8], b16, tag="dsp")
                nc.gpsimd.dma_start(
                    out=dsp[:, None, :],
                    in_=t[64:65, :].rearrange("p (a b) -> p a b", a=64),
                )
                rsp = work.tile([64, 8], b16, tag="rsp")
                with nc.allow_low_precision(reason="bf16 softmax recip"):
                    nc.vector.reciprocal(rsp[:], dsp[:])
                nc.gpsimd.dma_start(
                    out=rb[64:65, :].rearrange("p (a b) -> p a b", a=64),
                    in_=rsp[:, None, :],
                )

        def norm_head(c, j):
            te, to = aun[c]
            t, rb = (te, rbe) if j == 0 else (to, rbo)
            bc = psA.tile([128, 512], f32, tag="pp")
            nc.tensor.matmul(bc[:], lhsT=e64[:], rhs=rb[:], start=True, stop=True)
            bcs = work.tile([64, 512], b16, tag="bc")
            nc.vector.tensor_copy(bcs[:], bc[0:64, :])
            if j == 0:
                nc.vector.tensor_mul(atn[0:64, c, :], t[0:64, :], bcs[:])
            else:
                so = work.tile([64, 512], f8, tag="so")
                nc.vector.tensor_mul(so[:], t[0:64, :], bcs[:])
                nc.gpsimd.dma_start(out=atn[64:128, c, :], in_=so[:])
                aun.pop(c)

        def post_chunk2(c0, i):
            """One DoubleRow matmul: he-chunks c0,c0+1 into one PSUM tile."""
            sc, dc = i // 2, i % 2
            pp = psA.tile([128, 512], f32, tag="pp")
            nc.tensor.matmul(
                pp[:],
                lhsT=atn[:, c0:c0 + 2, sc * 128:(sc + 1) * 128],
                rhs=wp[:, c0:c0 + 2, dc * 512:(dc + 1) * 512],
                start=True,
                stop=True,
                perf_mode=DR,
            )
            dsl = slice(dc * 512, (dc + 1) * 512)
            if c0 == 0:
                nc.vector.tensor_copy(pacc[:, sc, dsl], pp[:])
            else:
                nc.vector.tensor_add(pacc[:, sc, dsl], pacc[:, sc, dsl], pp[:])

        def xres_preadd(i):
            """Fold the residual into pacc while exp still streams (pair 7)."""
            sc, dc = i // 2, i % 2
            dsl = slice(dc * 512, (dc + 1) * 512)
            nc.vector.tensor_add(pacc[:, sc, dsl], pacc[:, sc, dsl], xres[:, sc, dsl])

        def post_single(cx, i):
            """Single-chunk post matmul (K=128, no DoubleRow). For chunk 6 it
            accumulates into pacc during pair 7; for chunk 7 (the drain) it
            fuses the final add and streams the output out."""
            sc, dc = i // 2, i % 2
            pp = psA.tile([128, 512], f32, tag="pp")
            nc.tensor.matmul(
                pp[:],
                lhsT=atn[:, cx, sc * 128:(sc + 1) * 128],
                rhs=wp[:, cx, dc * 512:(dc + 1) * 512],
                start=True,
                stop=True,
            )
            dsl = slice(dc * 512, (dc + 1) * 512)
            if cx == 6:
                nc.vector.tensor_add(pacc[:, sc, dsl], pacc[:, sc, dsl], pp[:])
            else:
                ores = work.tile([128, 512], f32, tag="or")
                nc.vector.tensor_add(ores[:], pacc[:, sc, dsl], pp[:])
                dq = nc.sync if i % 2 == 0 else nc.gpsimd
                dq.dma_start(out=out_d[sc, :, dsl], in_=ores[:])

        exbs = {}

        def pair_blocks(c):
            """One pipeline step: scores/exp for pair c (row-tiled 64-row
            matmuls, heads on PE halves run concurrently), trailing PV for c,
            and the tail (PV-finish, normalize, post) of pair c-1."""
            g = c // 2
            for tcb in range(8):
                if c < 8:
                    if c == 0:
                        v_proj_tcc(2 * tcb)
                        v_proj_tcc(2 * tcb + 1)
                        if tcb in (0, 2, 4):  # K proj t-blocks 1..3 (group 0/1)
                            k_proj_tb(0, tcb // 2 + 1)
                    exb = work.tile([128, 2, 2, 512], f8, tag="exb", bufs=4)
                    tiles = []
                    for u in range(2):
                        tcc = 2 * tcb + u
                        ps2 = psS.tile([128, 2, 512], f32, tag="sc", name="ps2")
                        tiles.append((tcc, ps2))
                    # row-tiled pair: head 2c on PE rows 0-63, head 2c+1 on
                    # rows 64-127 -> the j=0/j=1 matmuls run concurrently
                    for tcc, ps2 in tiles:
                        nc.tensor.matmul(
                            ps2[:, 0, :],
                            lhsT=ktz[0:64, 2 * g, tcc * 128:(tcc + 1) * 128],
                            rhs=qt[0:64, c, :],
                            start=True,
                            stop=True,
                        )
                        nc.tensor.matmul(
                            ps2[:, 1, :],
                            lhsT=ktz[64:128, 2 * g + 1, tcc * 128:(tcc + 1) * 128],
                            rhs=qt[64:128, c, :],
                            start=True,
                            stop=True,
                        )
                    for u, (tcc, ps2) in enumerate(tiles):
                        nc.scalar.activation(exb[:, u, :, :], ps2[:], Exp)
                    exbs[(c, tcb)] = exb
                    if c == 0:
                        # AFTER this tcb's exps in the Scalar FIFO, so the
                        # transfers queue behind the critical input loads
                        late_loads(tcb)
                # tail of previous pair in fixed group slots
                if c > 0:
                    b = c - 1
                    if tcb == 0:
                        pv_mm(b, 6)
                    elif tcb == 1:
                        pv_mm(b, 7)
                        pv_evict(b, drain=(c == 8))
                    elif tcb == 2:
                        recips(b, drain=(c == 8))
                    elif tcb == 3:
                        norm_head(b, 0)
                    elif tcb == 4:
                        norm_head(b, 1)
                    elif tcb in (5, 6, 7) and c in (2, 4, 6):
                        for i in range((tcb - 5) * 3, min((tcb - 4) * 3, 8)):
                            post_chunk2(c - 2, i)
                if c == 7:
                    if tcb < 4:  # residual pre-add (pacc has chunks 0..5)
                        xres_preadd(2 * tcb)
                        xres_preadd(2 * tcb + 1)
                    elif tcb in (5, 6, 7):  # chunk 6 posts under the last exps
                        for i in range((tcb - 5) * 3, min((tcb - 4) * 3, 8)):
                            post_single(6, i)
                if c == 8 and tcb in (5, 6, 7):  # drain: chunk 7 + output
                    for i in range((tcb - 5) * 3, min((tcb - 4) * 3, 8)):
                        post_single(7, i)
                if c < 6 and tcb == (6 if c == 0 else 5):
                    q_proj(c + 2)
                # trailing PV for this pair (2 tcb behind)
                if c < 8 and tcb >= 2:
                    pv_mm(c, tcb - 2)

        def ores_out(i):
            sc, dc = i // 2, i % 2
            dsl = slice(dc * 512, (dc + 1) * 512)
            ores = work.tile([128, 512], f32, tag="or")
            eng = nc.vector if i % 2 == 0 else nc.gpsimd
            eng.tensor_add(ores[:], pacc[:, sc, dsl], xres[:, sc, dsl])
            dq = nc.sync if i % 2 == 0 else nc.gpsimd
            dq.dma_start(out=out_d[sc, :, dsl], in_=ores[:])

        k_proj_tb(0, 0)
        q_proj(0)
        pair_blocks(0)  # k_proj_tb(0, 1..3) run inside pair 0's slots
        q_proj(1)
        pair_blocks(1)
        for tb in range(4):
            k_proj_tb(1, tb)
        for c in range(2, 8):
            pair_blocks(c)
        pair_blocks(8)  # drain: tail of pair 7, posts for chunks 6,7, output


    nc.compile()
    return nc


def get_program():
    if "nc" not in _prog_cache:
        _prog_cache["nc"] = _build_program()
    return _prog_cache["nc"]


def _chunk128(a):
    n = a.shape[1]
    return np.ascontiguousarray(a.reshape(8, 128, n).transpose(1, 0, 2))


def make_in_maps(X, Wq, Wk, Wv, Wpost):
    X = np.asarray(X, dtype=np.float32)
    wq_p = _chunk128(np.asarray(Wq, dtype=np.float32)).astype(FP8)
    wk_p = _chunk128(np.asarray(Wk, dtype=np.float32)).astype(FP8)
    wv_p = _chunk128(np.asarray(Wv, dtype=np.float32)).astype(FP8)
    wp_p = _chunk128(np.asarray(Wpost, dtype=np.float32)).astype(FP8)

    xt_b = []
    for b in range(B):
        xt_b.append(_chunk128(np.ascontiguousarray(X[b].T)).astype(FP8))

    in_maps = []
    for core in range(NCORES):
        b = core // CORES_PER_BATCH
        q0 = (core % CORES_PER_BATCH) * SLOC
        xt = xt_b[b]
        xres = np.ascontiguousarray(
            X[b, q0:q0 + SLOC].reshape(4, 128, D).transpose(1, 0, 2)
        ).astype(BF16)
        in_maps.append(
            {
                "XT": xt,
                "XTQ": np.ascontiguousarray(xt[:, :, q0:q0 + SLOC]),
                "XRES": xres,
                "WQ": wq_p,
                "WK": wk_p,
                "WV": wv_p,
                "WP": wp_p,
            }
        )
    return in_maps


def assemble_output(results):
    out = np.empty((B, S, D), dtype=np.float32)
    for core, r in enumerate(results):
        b = core // CORES_PER_BATCH
        q0 = (core % CORES_PER_BATCH) * SLOC
        out[b, q0:q0 + SLOC] = np.asarray(r["OUT"]).reshape(SLOC, D)
    return out


def kernel(X, Wq, Wk, Wv, Wpost, _trace=False):
    from concourse.bass_utils import run_bass_kernel_spmd

    nc = get_program()
    in_maps = make_in_maps(X, Wq, Wk, Wv, Wpost)
    res = run_bass_kernel_spmd(nc, in_maps, core_ids=list(range(NCORES)), trace=_trace)
    out = assemble_output(res.results)
    if _trace:
        return out, res
    return out
